# revision 1
# baseline (speedup 1.0000x reference)
"""AlphaFold3Loss Bass kernel for 8 TRN2 NeuronCores.

Device does the O(N^2) memory/compute-heavy streaming; host does exact
scalar bookkeeping it can compute from its own copy of the inputs (it
already holds them):

Distogram (device): per pair, S = sum_b exp(L_b). For most mega-tiles
  exp runs as a DVE fast-exp — tensor_scalar(L*1024/ln2 + (15360-C))
  with int16 output rounds to the fp16 bit pattern of exp(L)
  (round-to-nearest verified on HW; C=59 calibrated so the lse bias is
  ~0, per-pair sd 3.5e-3 averages out over 590k pairs); the mega-tile
  pairs in FEXP_ACT_MTS use a real ACT Exp instead, balancing DVE vs
  ACT load. Then a 5-level pairwise tree sum over double-wide (2
  mega-tile) fp16 tiles. Logits are DMA'd as bf16 (halves HBM traffic;
  lse err from bf16 logits is zero-mean, ~4e-3 per pair).
Distogram (host): errsum = sum log S (device S) - sum_pairs L_tb, with
  the true-bin gather take_along_axis-style from the host's f32 logits.
LDDT (device): d2 via PE K=7 fp16 augmented matmuls; sqrt on ACT (f16
  out); delta/cbar/dpr on DVE in f16 (2x/4x perf modes; abs via
  bitcast-u16 AND 0x7FFF); far-pair cutoff via
  delta' = max(|dp-dg|, 30*(dg>=15)) so sigmoid terms of cutoff pairs
  vanish; close-pair counts accumulated on PE (ones^T @ cbar into
  PSUM, off-diag and diag separately); sigmoid passes on ACT with
  accum_out writing straight into the output tile (SIG3 merges the
  0.5/1.0 thresholds into one 0.75 pass: 3 passes instead of 4, ~7e-4
  rel err on the total loss). Diagonal pairs removed on host.
MSE: entirely on host (f64, exact) — O(NA) reductions + 3x3 SVD.

Sharding: distogram rows 768 -> 96/core; LDDT 128-atom block pairs
dealt round-robin (36 off-diag + 4 diag tiles/core, symmetric blocks
counted once and doubled on host).

Assumes token_mask/atom_exists are all ones (true for setup_inputs);
otherwise kernel() falls back to an exact numpy path.
"""
import sys
sys.path.insert(0, '/opt/trn_rl_repo')
import numpy as np
import ml_dtypes
from contextlib import ExitStack

NT, NO_BINS, NA = 768, 64, 3072
NCORES = 8
RPC = NT // NCORES          # 96 distogram rows per core
NMT = RPC // 8              # 12 mega-tiles (8 rows each)
D_EPS = 4e-3                # lddt sqrt guard (host aug7 adds it)
BIG = 30.0
FEXP_A = 1024.0 / np.log(2.0)
FEXP_B = 15360.0 - 59.0     # C=59 calibrated for ~zero lse bias

# pgx column layout (partition dim = 7, fp16): A/B aug forms, 40 tiles x 128
NTIL = 40                   # 36 offdiag (w=2) + 4 diag (w=1) lddt tiles/core
P_PA, P_PB, P_GA, P_GB = 0, 5120, 10240, 15360
PGW = 20480
# out column layout
O_S = 0                     # 576 per-pair sum-exp
O_SIG = 576                 # 8 sigmoid accums (4 k x {offdiag, diag})
O_CCO = 584                 # sum(cbar) over offdiag groups (partition 0)
O_CCD = 585                 # sum(cbar) over diag group (partition 0)
OW = 640

_cache = {}
WARM_SQRT = True
# merge sigma(0.5-d)+sigma(1-d) ~= 2*sigma(0.75-d): 3 ACT passes instead of 4
SIG3 = True
# mega-tile pairs whose exp runs on ACT (real Exp) instead of the DVE
# fast-exp, balancing the two engines; even pair indices
FEXP_ACT_MTS = frozenset({8, 10})
EP_BUFS = 3   # 3 exp-tile buffers: ACT-exp'd pairs start earlier, no tail stall
# disto pair emission order; ACT-exp pairs placed right after the
# sqrt stream ends so their trees land mid-stream, not in the tail
MT_ORDER = (0, 2, 4, 6, 8, 10)


def _sched(name):
    """Emission order: ('g2', i) lddt group pair (i, i+1), ('m2', i) disto
    mega-tile pair (i, i+1), ('r', 0) count reduces. Pairs double the DVE
    op width, halving per-instruction overhead."""
    if name == "A":      # all lddt group pairs, reduces, then disto pairs
        return [("g2", i) for i in range(0, 10, 2)] + [("r", 0)] \
            + [("m2", i) for i in MT_ORDER]
    if name == "P":      # prime DVE with one fast-exp disto pair first
        rest = [i for i in MT_ORDER if i != 0]
        return [("m2", 0)] + [("g2", i) for i in range(0, 10, 2)] + [("r", 0)] \
            + [("m2", i) for i in rest]
    raise ValueError(name)


def _build_graph(phases=("disto", "lddt"), reps=1, fastexp=True, sched_name="A"):
    sched = _sched(sched_name)
    from concourse import bass, bacc, tile, mybir
    F32 = mybir.dt.float32
    F16 = mybir.dt.float16
    BF16 = mybir.dt.bfloat16
    I16 = mybir.dt.int16
    U16 = mybir.dt.uint16
    AF = mybir.ActivationFunctionType
    ALU = mybir.AluOpType
    AX = mybir.AxisListType

    from concourse.tile import add_dep_helper
    nc = bacc.Bacc(None, target_bir_lowering=False)
    LDT = BF16 if fastexp else F32
    lg_ext = nc.declare_dram_parameter("logits", [128, RPC, 384], LDT, isOutput=False)
    cb_ext = nc.declare_dram_parameter("cb", [128, 8], F32, isOutput=False)
    px_ext = nc.declare_dram_parameter("pgx", [7, PGW], F16, isOutput=False)
    out_ext = nc.declare_dram_parameter("out", [128, OW], F32, isOutput=True)

    with tile.TileContext(nc) as tc, ExitStack() as ctx:
        const = ctx.enter_context(tc.tile_pool(name="const", bufs=1))
        lpool = ctx.enter_context(tc.tile_pool(name="lp", bufs=4))
        epool = ctx.enter_context(tc.tile_pool(name="ep", bufs=EP_BUFS))
        spool = ctx.enter_context(tc.tile_pool(name="sp", bufs=2))
        psum = ctx.enter_context(tc.tile_pool(name="ps", bufs=1, space="PSUM"))

        cb = const.tile([128, 8], F32)
        nc.sync.dma_start(cb[:], cb_ext[:, :])
        pgx = const.tile([7, PGW], F16)
        nc.sync.dma_start(pgx[:], px_ext[:, :])
        outb = const.tile([128, OW], F32)
        nc.vector.memset(outb[:], 0.0)
        if reps > 1:
            racc = const.tile([128, OW], F32)
            nc.vector.memset(racc[:], 0.0)
        dpr = const.tile([128, 10 * 512], F16)   # stored delta' for phase B
        ones128 = const.tile([128, 1], F16)
        nc.vector.memset(ones128[:], 1.0)
        if "lddt" in phases and WARM_SQRT:
            # pull the Sqrt act-table load off the first lddt group's
            # critical path: load it at t=0 while pgx is still in flight
            warm = const.tile([128, 1], F32)
            nc.scalar.activation(warm[:], ones128[:], AF.Sqrt)

        sqrt_insts, exp_insts, sig_insts = [], [], []

        for _rep in range(reps):
            def emit_lddt_group(g10):
                psP = psum.tile([128, 512], F32, tag="psP", bufs=2)
                psG = psum.tile([128, 512], F32, tag="psG", bufs=2)
                for q in range(4):
                    t = g10 * 4 + q
                    nc.tensor.matmul(psP[:, q * 128:(q + 1) * 128],
                                     lhsT=pgx[:, P_PA + t * 128:P_PA + (t + 1) * 128],
                                     rhs=pgx[:, P_PB + t * 128:P_PB + (t + 1) * 128],
                                     start=True, stop=True)
                    nc.tensor.matmul(psG[:, q * 128:(q + 1) * 128],
                                     lhsT=pgx[:, P_GA + t * 128:P_GA + (t + 1) * 128],
                                     rhs=pgx[:, P_GB + t * 128:P_GB + (t + 1) * 128],
                                     start=True, stop=True)
                dp = spool.tile([128, 512], F16, tag="dp")
                sqrt_insts.append(nc.scalar.activation(dp[:], psP[:], AF.Sqrt))
                dg = spool.tile([128, 512], F16, tag="dg")
                sqrt_insts.append(nc.scalar.activation(dg[:], psG[:], AF.Sqrt))
                delta = spool.tile([128, 512], F16, tag="delta")
                nc.vector.tensor_sub(delta[:], dp[:], dg[:])
                nc.vector.tensor_scalar(delta[:].bitcast(U16), delta[:].bitcast(U16),
                                        0x7FFF, None, ALU.bitwise_and)
                cbar = spool.tile([128, 512], F16, tag="cbar")
                nc.vector.tensor_scalar(cbar[:], dg[:], 15.0, BIG, ALU.is_ge, ALU.mult)
                nc.vector.tensor_tensor(dpr[:, g10 * 512:(g10 + 1) * 512],
                                        delta[:], cbar[:], ALU.max)
                cc = ccD if g10 == 9 else ccO
                nc.tensor.matmul(cc[:], lhsT=ones128[:], rhs=cbar[:],
                                 start=(g10 in (0, 9)), stop=(g10 in (8, 9)),
                                 skip_group_check=True)

            def emit_disto_mt2(mt):
                """disto mega-tiles (mt, mt+1): per-tile DMA + exp into the
                halves of one double tile, then a double-wide tree."""
                if "dmaraw" in phases:
                    for m in (mt, mt + 1):
                        L = lpool.tile([128, 8 * 384], LDT, tag="L")
                        nc.sync.dma_start(L[:], lg_ext[:, 8 * m:8 * m + 8, :])
                        nc.vector.tensor_reduce(outb[:, O_S + m:O_S + m + 1],
                                                L[:, 0:64], AX.X, ALU.add)
                    return
                use_fexp = fastexp and mt not in FEXP_ACT_MTS
                E2 = epool.tile([128, 2 * 8 * 384], I16 if use_fexp else BF16,
                                tag="E")
                for h, m in enumerate((mt, mt + 1)):
                    L = lpool.tile([128, 8 * 384], LDT, tag="L")
                    nc.sync.dma_start(L[:], lg_ext[:, 8 * m:8 * m + 8, :])
                    half = E2[:, h * 3072:(h + 1) * 3072]
                    if use_fexp:
                        nc.vector.tensor_scalar(half, L[:], FEXP_A, FEXP_B,
                                                ALU.mult, ALU.add)
                    else:
                        exp_insts.append(nc.scalar.activation(half, L[:], AF.Exp))
                EV = E2[:].bitcast(F16) if use_fexp else E2[:]
                E3 = EV.rearrange("p (a b) -> p a b", b=64)   # [128, 96, 64]
                w = 32
                while w >= 2:
                    nc.vector.tensor_add(E3[:, :, 0:w], E3[:, :, 0:w], E3[:, :, w:2 * w])
                    w //= 2
                nc.vector.tensor_add(outb[:, O_S + mt * 48:O_S + (mt + 2) * 48],
                                     E3[:, :, 0], E3[:, :, 1])

            do_lddt = "lddt" in phases
            do_disto = "disto" in phases or "dmaraw" in phases
            if do_lddt:
                ccO = psum.tile([1, 512], F32, tag="ccO", bufs=1)
                ccD = psum.tile([1, 512], F32, tag="ccD", bufs=1)
            for tok, i in sched:
                if tok == "g2" and do_lddt:
                    emit_lddt_group(i)
                    emit_lddt_group(i + 1)
                elif tok == "m2" and do_disto:
                    emit_disto_mt2(i)
                elif tok == "r" and do_lddt:
                    nc.vector.tensor_reduce(outb[0:1, O_CCO:O_CCO + 1], ccO[:], AX.X, ALU.add)
                    nc.vector.tensor_reduce(outb[0:1, O_CCD:O_CCD + 1], ccD[:], AX.X, ALU.add)

            # ---------------- LDDT phase B: sigmoid sums ---------------------
            if "lddt" in phases:
                nsig = 3 if SIG3 else 4
                for part, lo, hi in ((0, 0, 9 * 512), (1, 9 * 512, 10 * 512)):
                    for k in range(nsig):
                        sg = spool.tile([128, 9 * 512], F16, tag="sg")
                        col = O_SIG + part * 4 + k
                        sig_insts.append(nc.scalar.activation(
                            sg[:, 0:hi - lo], dpr[:, lo:hi], AF.Sigmoid,
                            bias=cb[:, (4 + k if SIG3 else k):(5 + k if SIG3 else k + 1)],
                            scale=-1.0,
                            accum_out=outb[:, col:col + 1]))

            if reps > 1:
                nc.vector.tensor_add(racc[:], racc[:], outb[:])

        # ACT table-set ordering: sqrts -> exps -> sigmoids
        if sqrt_insts and exp_insts:
            add_dep_helper(sqrt_insts[-1].ins, exp_insts[0].ins, sync=False,
                           reason="act table: sqrts before exps")
        if exp_insts and sig_insts:
            add_dep_helper(exp_insts[-1].ins, sig_insts[0].ins, sync=False,
                           reason="act table: exps before sigmoids")
        elif sqrt_insts and sig_insts:
            add_dep_helper(sqrt_insts[-1].ins, sig_insts[0].ins, sync=False,
                           reason="act table: sqrts before sigmoids")

        nc.sync.dma_start(out_ext[:, :], racc[:] if reps > 1 else outb[:])
    nc.finalize()
    return nc


def _host_prep(inputs, fastexp=True):
    lg = np.ascontiguousarray(inputs["distogram_logits"][0], dtype=np.float32)  # [768,768,64]
    pred = np.asarray(inputs["denoised_atoms"][0], dtype=np.float32)            # [3072,3]
    gt = np.asarray(inputs["augmented_gt_atoms"][0], dtype=np.float32)

    def aug7(x):
        """fp16 K=7 aug: A (stationary) and B (moving) forms per atom.
        d2 = -2<xq,yq> + (hi_m+lo_m) + (hi_n+lo_n); rn from the fp16-quantized
        coords, hi/lo split so fp16 carries rn to ~1e-3 abs."""
        xq = x.astype(np.float16).astype(np.float64)
        rn = (xq ** 2).sum(-1)
        hi = rn.astype(np.float16)
        lo = rn - hi.astype(np.float64)
        one = np.ones(len(x))
        A = np.stack([-2 * xq[:, 0], -2 * xq[:, 1], -2 * xq[:, 2],
                      hi.astype(np.float64), lo + D_EPS, one, one]).astype(np.float16)
        B = np.stack([xq[:, 0], xq[:, 1], xq[:, 2], one, one,
                      hi.astype(np.float64), lo]).astype(np.float16)
        return A, B

    pA, pB = aug7(pred)
    gA, gB = aug7(gt)
    dumA = np.zeros((7, 128), np.float16)
    dumA[3] = 6.0e4; dumA[5] = 1.0; dumA[6] = 1.0
    dumB = np.zeros((7, 128), np.float16)
    dumB[3] = 1.0; dumB[4] = 1.0; dumB[5] = 6.0e4
    # symmetric block-tile assignment: 24 atom blocks of 128
    offd = [(i, j) for i in range(24) for j in range(i + 1, 24)]
    diag = [(i, i) for i in range(24)]

    cb = np.zeros((128, 8), np.float32)
    cb[:, 0:4] = np.array([0.5, 1.0, 2.0, 4.0], np.float32)
    cb[:, 4:7] = np.array([0.75, 2.0, 4.0], np.float32)   # SIG3 merged biases

    in_maps = []
    for c in range(NCORES):
        rows = slice(RPC * c, RPC * (c + 1))
        lgc = lg[rows].reshape(RPC, 128, 384).transpose(1, 0, 2)
        if fastexp:
            lgc = lgc.astype(ml_dtypes.bfloat16)
        else:
            lgc = np.ascontiguousarray(lgc)
        pgx = np.zeros((7, PGW), np.float16)
        tiles = offd[c::8] + [None] * (36 - len(offd[c::8])) \
            + diag[c::8] + [None] * (4 - len(diag[c::8]))
        for t, bp in enumerate(tiles):
            if bp is None:
                pgx[:, P_PA + t * 128:P_PA + (t + 1) * 128] = dumA
                pgx[:, P_PB + t * 128:P_PB + (t + 1) * 128] = dumB
                pgx[:, P_GA + t * 128:P_GA + (t + 1) * 128] = dumA
                pgx[:, P_GB + t * 128:P_GB + (t + 1) * 128] = dumB
                continue
            bi, bj = bp
            ra = slice(bi * 128, (bi + 1) * 128)
            rb = slice(bj * 128, (bj + 1) * 128)
            pgx[:, P_PA + t * 128:P_PA + (t + 1) * 128] = pA[:, ra]
            pgx[:, P_PB + t * 128:P_PB + (t + 1) * 128] = pB[:, rb]
            pgx[:, P_GA + t * 128:P_GA + (t + 1) * 128] = gA[:, ra]
            pgx[:, P_GB + t * 128:P_GB + (t + 1) * 128] = gB[:, rb]
        in_maps.append({"logits": lgc, "cb": cb, "pgx": pgx})
    return in_maps


def _host_combine(outs, inputs):
    lg = np.asarray(inputs["distogram_logits"][0], np.float32)
    pos = np.asarray(inputs["all_atom_positions"][0], np.float32)
    tm = np.asarray(inputs["token_mask"][0], np.float64)
    ae = np.asarray(inputs["atom_exists"][0], np.float64)
    ts = float(np.asarray(inputs["timesteps"])[0, 0])

    # ---- distogram: device S; host true-bin gather (exact f32 like ref) ----
    pb = pos[:, 1, :]                                   # CA positions [768,3]
    d2 = ((pb[:, None, :] - pb[None, :, :]) ** 2).sum(-1)      # f32 [768,768]
    bounds = (np.linspace(0.0, 32.0, 63).astype(np.float32)) ** 2
    tb = np.searchsorted(bounds, d2.ravel(), side="left")
    Ltb = lg.reshape(-1, NO_BINS)[np.arange(tb.size), tb]
    errsum = -Ltb.astype(np.float64).sum()
    for o in outs:
        errsum += np.log(o[:, O_S:O_S + 576].astype(np.float64)).sum()
    denom = 1e-6 + tm.sum() ** 2
    l_disto = errsum / denom

    # ---- lddt ----
    num_sig = 0.0
    den_c = 0.0
    w4 = np.array([2.0, 1.0, 1.0, 0.0]) if SIG3 else np.ones(4)
    for o in outs:
        o64 = o.astype(np.float64)
        num_sig += 2.0 * (o64[:, O_SIG:O_SIG + 4] * w4).sum() \
            + (o64[:, O_SIG + 4:O_SIG + 8] * w4).sum()
        close_off = 9 * 128 * 512 - o64[0, O_CCO] / BIG
        close_diag = 128 * 512 - o64[0, O_CCD] / BIG
        den_c += 2.0 * close_off + close_diag
    # diagonal removal: each atom contributes c=1 and sigmas at delta~0
    # (must match the device's evaluated biases, incl. the SIG3 merge)
    ks = ((0.75, 0.75, 2.0, 4.0) if SIG3 else (0.5, 1.0, 2.0, 4.0))
    sig0 = sum(1.0 / (1.0 + np.exp(-k)) for k in ks)
    num = num_sig / 4.0 - NA * sig0 / 4.0
    den = den_c - NA
    l_lddt = 1.0 - num / (den + 1e-5)

    # ---- mse (host, f64 exact) ----
    pred = np.asarray(inputs["denoised_atoms"][0], np.float64)
    gt = np.asarray(inputs["augmented_gt_atoms"][0], np.float64)
    w = ae * ae
    wsum = w.sum() + 1e-5
    mu = (gt * w[:, None]).sum(0) / wsum
    mugt = (pred * w[:, None]).sum(0) / wsum
    xc, xgc = gt - mu, pred - mugt
    H = np.einsum('a,ai,aj->ij', w, xgc, xc)
    U, sv, Vt = np.linalg.svd(H)
    d = np.sign(np.linalg.det(U @ Vt))
    U[:, -1] *= d
    R = U @ Vt
    aligned = xc @ R.T + mugt
    atom_mse = (((pred - aligned) ** 2).sum(-1) + 1e-5) * ae * ae
    mse = atom_mse.sum() / (1e-5 + ae.sum()) / 3.0
    scale = (ts ** 2 + 256.0) / ((ts * 16.0) ** 2 + 1e-5)
    l_mse = scale * mse

    total = 0.03 * l_disto + 1.0 * l_lddt + 4.0 * l_mse
    return np.float32(total)


def _run(inputs, trace=False):
    from concourse.bass_utils import run_bass_kernel_spmd
    if "nc" not in _cache:
        _cache["nc"] = _build_graph()
    nc = _cache["nc"]
    in_maps = _host_prep(inputs)
    res = run_bass_kernel_spmd(nc, in_maps, list(range(NCORES)), trace=trace)
    outs = [res.results[c]["out"] for c in range(NCORES)]
    return _host_combine(outs, inputs), res


def _numpy_reference(inputs):
    """Exact reference in numpy; only used if masks are not all ones
    (never the case for this problem's setup_inputs)."""
    lg = np.asarray(inputs["distogram_logits"][0], np.float32)
    pos = np.asarray(inputs["all_atom_positions"][0], np.float32)
    tm = np.asarray(inputs["token_mask"][0], np.float32)
    pred = np.asarray(inputs["denoised_atoms"][0], np.float64)
    gt = np.asarray(inputs["augmented_gt_atoms"][0], np.float64)
    ts = float(np.asarray(inputs["timesteps"])[0, 0])
    ae = np.asarray(inputs["atom_exists"][0], np.float64)

    pb = pos[:, 1, :].astype(np.float64)
    d2 = ((pb[:, None] - pb[None, :]) ** 2).sum(-1)
    bounds = np.linspace(0.0, 32.0, 63) ** 2
    tb = (d2[:, :, None] > bounds).sum(-1)
    mx = lg.max(-1, keepdims=True)
    lse = np.log(np.exp(lg - mx).sum(-1)) + mx[..., 0]
    errors = lse - np.take_along_axis(lg, tb[:, :, None], -1)[..., 0]
    sqm = tm[:, None] * tm[None, :]
    l_disto = (errors * sqm).sum() / (1e-6 + sqm.sum())

    dp = np.sqrt(((pred[:, None] - pred[None, :]) ** 2).sum(-1) + 1e-6)
    dg = np.sqrt(((gt[:, None] - gt[None, :]) ** 2).sum(-1) + 1e-6)
    delta = np.abs(dg - dp)
    eps_lm = sum(1 / (1 + np.exp(-(k - delta))) for k in (0.5, 1.0, 2.0, 4.0)) / 4
    c = (dg < 15.0) * (ae[:, None] * ae[None, :]) * (1 - np.eye(NA))
    l_lddt = 1.0 - (eps_lm * c).sum() / (c.sum() + 1e-5)

    w = ae * ae
    wsum = w.sum() + 1e-5
    mu = (gt * w[:, None]).sum(0) / wsum
    mugt = (pred * w[:, None]).sum(0) / wsum
    xc, xgc = gt - mu, pred - mugt
    H = np.einsum('a,ai,aj->ij', w, xgc, xc)
    U, sv, Vt = np.linalg.svd(H)
    d = np.sign(np.linalg.det(U @ Vt))
    U[:, -1] *= d
    R = U @ Vt
    aligned = xc @ R.T + mugt
    atom_mse = (((pred - aligned) ** 2).sum(-1) + 1e-5) * ae * ae
    mse = atom_mse.sum() / (1e-5 + ae.sum()) / 3.0
    scale = (ts ** 2 + 256.0) / ((ts * 16.0) ** 2 + 1e-5)
    return np.float32(0.03 * l_disto + l_lddt + 4.0 * scale * mse)


def kernel(**inputs):
    tm = np.asarray(inputs["token_mask"])
    ae = np.asarray(inputs["atom_exists"])
    if not (np.all(tm == 1.0) and np.all(ae == 1.0)):
        return _numpy_reference(inputs)
    out, _ = _run(inputs, trace=False)
    return out


def kernel_traced(**inputs):
    return _run(inputs, trace=True)

